# revision 1
# baseline (speedup 1.0000x reference)
"""Causal multi-head attention (B=2, L=2048, D=1024, H=16) on 8 trn2 cores.

Sharding: DP on batch (2) x TP on heads (4 groups of 4 heads) = 8 cores.
Each core computes, for its (batch b, head-group g):
  - qT/kT = wqk_g^T @ x_b^T            [512, L]   (head dims on partitions)
  - V     = x_b @ wv_g (+ ones cols)   [L, 4*65]  (natural layout, per-head ones
                                                   column so the PV matmul also
                                                   produces softmax denominators)
  - S^T   = K Q^T per (k-block, q-tile), causal-trimmed, both heads of a
            pair row-packed into one concurrent PE pass; exp on ACT
            (single 3D-AP call on diagonal blocks); multiplicative
            triangular mask (idle GpSimd) on diagonal blocks
  - out^T = V_ext^T @ E^T accumulated over k-blocks  -> PSUM
            (partition 64 resp. 32 holds the softmax denominator r)
  - attn^T: unnormalized out^T staged to SBUF (DVE) while the r rows are
            partition-broadcast straight from PSUM via step-0-free-dim
            DMAs (frees the PSUM accumulator within the pipeline-fill
            window); 1/r = exp(-ln r) on ACT using the combined
            natural_log_exp table set (no per-q-tile table reloads);
            in-place DVE multiply
  - y_part = attn @ w_out[rows of g]   [L, 1024]  (row-parallel out-proj),
            interleaved one q-tile behind attention so out-proj matmuls +
            y DMA hide under the ACT-bound attention phase
Host gathers: y_b = sum_g y_part + (b_qkv_v @ w_out + b_out).

All matmuls run in bf16 (inputs host-rounded): full-rate PE streaming,
PV lhsT padded to 128 cols (FWL-eligible), and PV emission skewed behind
scores/exp so the PE FIFO never head-of-line blocks on ACT.
"""

import sys
from contextlib import ExitStack

if "/opt/trn_rl_repo" not in sys.path:
    sys.path.insert(0, "/opt/trn_rl_repo")

import ml_dtypes
import numpy as np

import concourse.bass as bass
import concourse.mybir as mybir
import concourse.tile as tile
from concourse import bacc
from concourse.bass import ts
from concourse.bass_utils import run_bass_kernel_spmd

F32 = mybir.dt.float32
BF16 = mybir.dt.bfloat16
AF = mybir.ActivationFunctionType
OP = mybir.AluOpType

B, D, H = 2, 1024, 16
HD = 64           # head dim
NH = 4            # heads per core
GD = NH * HD      # 256 head dims per core
P = 128
QTW = 512         # q-tile width
VSTR = 193        # per-pair stride in the v tile: [V0(64)|1] + [z32|1|z31|V1(64)]
VW = 2 * VSTR     # v tile width (2 pairs)

def bcast_ap(row_ap, n_part):
    """[1, N] AP -> (1, n_part, N) AP replicating the row (step-0 free
    dim), for DMA partition-broadcast."""
    from concourse.ap import AP

    dims = list(row_ap.ap)
    assert dims[0][1] == 1 and len(dims) == 2, dims
    return AP(row_ap.tensor, row_ap.offset,
              [list(dims[0]), [0, n_part], list(dims[1])])


class _combined_exp_ln_tables:
    """Make the ACT table-load pass pick `natural_log_exp_and_others` for
    both Exp and Ln (it exists in act_info.json with full 400-bucket
    splines for each) instead of ping-ponging between the exp-only and
    ln-only sets.  Set names/ids are untouched; Exp/Ln are only removed
    from the *other* sets' membership used by the placement analysis."""

    def __enter__(self):
        self._orig = bacc.get_activation_tables
        combined = {AF.Exp, AF.Ln}

        def patched(arch):
            tabs = self._orig(arch)
            out = {}
            for name, funcs in tabs.items():
                if name != "natural_log_exp_and_others":
                    funcs = funcs - combined
                out[name] = funcs
            return out

        bacc.get_activation_tables = patched
        return self

    def __exit__(self, *exc):
        bacc.get_activation_tables = self._orig


def build_nc(L=2048):
    """Build the per-core Bass program. Same program for all 8 cores (SPMD)."""
    DK = D // P       # 8 contraction chunks
    LT = L // P       # l-tiles
    QT = L // QTW     # q-tiles
    QB = QTW // P     # k-blocks per q-tile (4)

    nc = bacc.Bacc("TRN2", target_bir_lowering=False, debug=False, num_devices=8)

    xT = nc.dram_tensor("xT", [D, L], BF16, kind="ExternalInput").ap()
    wqk = nc.dram_tensor("wqk", [D, 2 * GD], BF16, kind="ExternalInput").ap()
    wv = nc.dram_tensor("wv", [D, GD], BF16, kind="ExternalInput").ap()
    wo = nc.dram_tensor("wo", [GD, D], BF16, kind="ExternalInput").ap()
    bqk = nc.dram_tensor("bqk", [2 * GD, 1], F32, kind="ExternalInput").ap()
    mask = nc.dram_tensor("mask", [P, P], BF16, kind="ExternalInput").ap()
    # ones/zeros filler for the V slots: [1, 0*32, 1, 0*31] per partition
    vpat = nc.dram_tensor("vpat", [P, 65], BF16, kind="ExternalInput").ap()
    y = nc.dram_tensor("y", [L, D], F32, kind="ExternalOutput").ap()

    with tile.TileContext(nc) as tc, ExitStack() as stk:
        # ---------- persistent pools ----------
        const = stk.enter_context(tc.tile_pool(name="const", bufs=1))
        qk_pool = stk.enter_context(tc.tile_pool(name="qk", bufs=1))
        v_pool = stk.enter_context(tc.tile_pool(name="v", bufs=1))
        attn_pool = stk.enter_context(tc.tile_pool(name="attn", bufs=1))
        wo_pool = stk.enter_context(tc.tile_pool(name="wo", bufs=1))

        bqk_sb = const.tile([P, 4], F32, tag="bqk", name="bqk_sb")
        for m in range(4):
            nc.sync.dma_start(bqk_sb[:, m : m + 1], bqk[ts(m, P)])
        mask_sb = const.tile([P, P], BF16, tag="mask", name="mask_sb")
        nc.sync.dma_start(mask_sb[:], mask)
        vpat_sb = const.tile([P, 65], BF16, tag="vpat", name="vpat_sb")
        nc.sync.dma_start(vpat_sb[:], vpat)

        # m-tile 0,1 = qT (head pairs 01, 23); 2,3 = kT
        qk_sb = [qk_pool.tile([P, L], BF16, tag=f"qk{m}", name=f"qk_sb{m}") for m in range(4)]
        v_sb = [v_pool.tile([P, VW], BF16, tag=f"v{t}", name=f"v_sb{t}") for t in range(LT)]
        attn_sb = [attn_pool.tile([P, L], BF16, tag=f"attn{p}", name=f"attn_sb{p}") for p in range(2)]
        wo_sb = [wo_pool.tile([P, D], BF16, tag=f"wo{c}", name=f"wo_sb{c}") for c in range(2)]
        for c in range(2):
            for h in range(2):
                nc.sync.dma_start(wo_sb[c][:, ts(h, QTW)], wo[ts(c, P), ts(h, QTW)])

        # constant [ones|zeros|ones|zeros] filler at cols 64:129 per pair
        # (written once per persistent v tile, via idle GpSimd, not 32 DMAs)
        for lt in range(LT):
            vv = v_sb[lt][:, 0:VW].rearrange("p (a c) -> p a c", a=2, c=VSTR)
            for a in range(2):
                nc.gpsimd.tensor_copy(vv[:, a, 64:129], vpat_sb[:])

        # ---------- phase 1+2: projections ----------
        with (
            tc.tile_pool(name="xt", bufs=1) as xt_pool,
            tc.tile_pool(name="wi", bufs=1) as wi_pool,
            tc.tile_pool(name="psp", bufs=1, space="PSUM") as psp,
            tc.tile_pool(name="psv", bufs=4, space="PSUM") as psv,
        ):
            xt_sb = [xt_pool.tile([P, L], BF16, tag=f"xt{k}", name=f"xt_sb{k}") for k in range(DK)]
            wqk_sb = [wi_pool.tile([P, 2 * GD], BF16, tag=f"wqk{k}", name=f"wqk_sb{k}") for k in range(DK)]
            wv_sb = [wi_pool.tile([P, GD], BF16, tag=f"wv{k}", name=f"wv_sb{k}") for k in range(DK)]
            # first halves of all xT chunks land first (2KB descriptors,
            # 16 DMAs spread over the queues): v-proj starts after ~10us
            for k in range(DK):
                nc.sync.dma_start(wv_sb[k][:], wv[ts(k, P)])
            for half in range(2):
                for k in range(DK):
                    nc.sync.dma_start(
                        xt_sb[k][:, ts(half, L // 2)],
                        xT[ts(k, P), ts(half, L // 2)])
            for k in range(DK):
                nc.sync.dma_start(wqk_sb[k][:], wqk[ts(k, P)])

            # V natural: [L, 256] = x @ wv, packed into per-head [V|ones] slots
            for lt in range(LT):
                vt = v_sb[lt]
                vv = vt[:, 0:VW].rearrange("p (a c) -> p a c", a=2, c=VSTR)
                ps = psv.tile([P, GD], F32, tag="psv", name="ps_v")
                for k in range(DK):
                    nc.tensor.matmul(
                        ps[:],
                        xt_sb[k][:, ts(lt, P)],
                        wv_sb[k][:],
                        start=(k == 0),
                        stop=(k == DK - 1),
                    )
                pv = ps[:].rearrange("p (a c) -> p a c", a=2, c=2 * HD)
                nc.vector.tensor_copy(vv[:, :, 0:64], pv[:, :, 0:64])      # heads 0,2
                nc.vector.tensor_copy(vv[:, :, 129:193], pv[:, :, 64:128])  # heads 1,3
            # qT/kT: [512, L] = wqk^T @ xT, bias added during PSUM eviction.
            # k-outer over 4 accumulating banks: each weight tile is loaded
            # once and streamed against all 4 L-slices (fewer LDWEIGHTS, no
            # PE micro-gaps).
            NT = L // QTW
            for m in (0, 2, 1, 3):
                ps4 = [psp.tile([P, QTW], F32, tag=f"psp{n}", name=f"ps_p{n}")
                       for n in range(NT)]
                for k in range(DK):
                    for n in range(NT):
                        nc.tensor.matmul(
                            ps4[n][:],
                            wqk_sb[k][:, ts(m, P)],
                            xt_sb[k][:, ts(n, QTW)],
                            start=(k == 0),
                            stop=(k == DK - 1),
                        )
                for n in range(NT):
                    nc.vector.tensor_scalar(
                        out=qk_sb[m][:, ts(n, QTW)],
                        in0=ps4[n][:],
                        scalar1=bqk_sb[:, m : m + 1],
                        scalar2=None,
                        op0=OP.add,
                    )

        # ---------- phase 3: attention + interleaved out-projection ----------
        # One group per k-block covering BOTH heads of the pair:
        #   scores row-packed (head0 -> PE rows 0-63, head1 -> rows 64-127,
        #   concurrent), one exp over both banks, PV per head.
        # PV emission is skewed behind scores/exp so the PE FIFO never
        # head-of-line blocks on ACT.  Out-projection for q-tile qt is
        # emitted after attention pair 0 of q-tile qt+1, giving its inputs
        # (both pairs' normalized attn) a full sub-phase of slack.
        with (
            tc.tile_pool(name="e", bufs=4) as e_pool,
            tc.tile_pool(name="sc", bufs=2) as sc_pool,
            tc.tile_pool(name="bc", bufs=2) as bc_pool,
            tc.tile_pool(name="ysb", bufs=3) as y_pool,
            tc.tile_pool(name="pss", bufs=2, space="PSUM") as pss,
            tc.tile_pool(name="pso", bufs=1, space="PSUM") as pso,
            tc.tile_pool(name="psy", bufs=1, space="PSUM") as psy_pool,
        ):
            SKEW = 3

            def attention(pair, qt, inject=None):
                q_t = qk_sb[pair]
                k_t = qk_sb[2 + pair]
                out_ps = pso.tile([P, 2 * QTW], F32, tag="pso", name="ps_o")
                nblk = QB * qt + QB     # k-blocks for this q-tile

                def front(j):
                    """Row-packed scores + exp (+ masks) for k-block j.
                    Returns a closure emitting the two PV matmuls."""
                    sp = pss.tile([P, 2 * QTW], F32, tag="pss", name="ps_s")
                    e_t = e_pool.tile([P, 2 * QTW], BF16, tag="e", name="e_t")
                    diag = j >= QB * qt
                    da = (j - QB * qt) * P if diag else 0
                    for hl in range(2):
                        hb = 64 * hl
                        nc.tensor.matmul(
                            sp[:, hl * QTW + da : (hl + 1) * QTW],
                            k_t[hb : hb + 64, ts(j, P)],
                            q_t[hb : hb + 64,
                                qt * QTW + da : (qt + 1) * QTW],
                            start=True, stop=True)
                    if da == 0:
                        nc.scalar.activation(e_t[:], sp[:], AF.Exp,
                                             scale=0.125)
                    else:
                        # single ACT call over both heads' [da:QTW] spans
                        spv = sp[:].rearrange("p (a c) -> p a c", a=2, c=QTW)
                        ev = e_t[:].rearrange("p (a c) -> p a c", a=2, c=QTW)
                        nc.scalar.activation(ev[:, :, da:QTW],
                                             spv[:, :, da:QTW],
                                             AF.Exp, scale=0.125)
                    if diag:  # triangular masks on idle GpSimd
                        nc.gpsimd.tensor_tensor(
                            out=e_t[:, da : da + P],
                            in0=e_t[:, da : da + P],
                            in1=mask_sb[:], op=OP.mult)
                        nc.gpsimd.tensor_tensor(
                            out=e_t[:, QTW + da : QTW + da + P],
                            in0=e_t[:, QTW + da : QTW + da + P],
                            in1=mask_sb[:], op=OP.mult)

                    def emit_pv(j=j, da=da, e_t=e_t):
                        for hl in range(2):
                            nc.tensor.matmul(
                                out_ps[:, hl * QTW + da : (hl + 1) * QTW],
                                vext(v_sb[j], pair, hl),
                                e_t[:, hl * QTW + da : (hl + 1) * QTW],
                                start=(j == 0), stop=(j == nblk - 1))
                    return emit_pv

                pend = []
                for j in range(nblk):
                    pend.append(front(j))
                    if j >= SKEW:
                        pend[j - SKEW]()
                    if inject is not None:
                        inject(j)
                for j in range(max(0, nblk - SKEW), nblk):
                    pend[j]()

                # stage unnormalized out^T (incl. the r rows 64/32) to SBUF:
                # frees out_ps (bufs=1) within the next sub-phase's
                # pipeline-fill window.  r rows go first so the 1/r
                # broadcast (deferred normfinish) can start ASAP.
                sc = sc_pool.tile([P, 2 * QTW], F32, tag="sc", name="sc_t")
                nc.vector.tensor_copy(sc[64:65, 0:QTW], out_ps[64:65, 0:QTW])
                nc.vector.tensor_copy(sc[32:33, QTW : 2 * QTW],
                                      out_ps[32:33, QTW : 2 * QTW])
                nc.vector.tensor_copy(sc[0:64, 0:QTW], out_ps[0:64, 0:QTW])
                nc.vector.tensor_copy(sc[64:P, QTW : 2 * QTW],
                                      out_ps[64:P, QTW : 2 * QTW])
                return sc

            def normfinish(sc, pair, qt):
                # 1/r broadcast + exp(-ln r) on ACT (combined table set, no
                # reload) + DVE normalize into the bf16 attn tile.  Emitted
                # a few passes into the NEXT sub-phase so nothing here ever
                # head-of-line blocks an engine FIFO.
                bc = bc_pool.tile([P, QTW], F32, tag="bc", name="bc_t")
                for h in range(2):
                    nc.sync.dma_start(
                        bc[32 * h : 32 * (h + 1), :],
                        bcast_ap(sc[64:65, 0:QTW], 32))
                    nc.sync.dma_start(
                        bc[64 + 32 * h : 96 + 32 * h, :],
                        bcast_ap(sc[32:33, QTW : 2 * QTW], 32))
                nc.scalar.activation(bc[:], bc[:], AF.Ln)
                nc.scalar.activation(bc[:], bc[:], AF.Exp, scale=-1.0)
                nc.vector.tensor_tensor(
                    out=attn_sb[pair][0:64, ts(qt, QTW)],
                    in0=sc[0:64, 0:QTW], in1=bc[0:64, :], op=OP.mult)
                nc.vector.tensor_tensor(
                    out=attn_sb[pair][64:P, ts(qt, QTW)],
                    in0=sc[64:P, QTW : 2 * QTW], in1=bc[64:P, :], op=OP.mult)

            def outproj2(qt, half):
                # out-projection, two l-tiles at a time (both pairs done)
                for lt in range(QB * qt + 2 * half, QB * qt + 2 * half + 2):
                    ps2 = [psy_pool.tile([P, QTW], F32, tag=f"psy{nh}",
                                         name=f"ps_y{nh}") for nh in range(2)]
                    for c in range(2):  # attn weight tile reused over halves
                        for nh in range(2):
                            nc.tensor.matmul(
                                ps2[nh][:],
                                attn_sb[c][:, ts(lt, P)],
                                wo_sb[c][:, ts(nh, QTW)],
                                start=(c == 0),
                                stop=(c == 1),
                            )
                    yt = y_pool.tile([P, D], F32, tag="y", name="y_t")
                    for nh in range(2):
                        nc.vector.tensor_copy(yt[:, ts(nh, QTW)], ps2[nh][:])
                    for h in range(2):  # row-split keeps 4KB descriptors
                        nc.sync.dma_start(
                            y[lt * P + 64 * h : lt * P + 64 * (h + 1), :],
                            yt[64 * h : 64 * (h + 1), :])

            scs = {}
            for qt in range(QT):
                scs[0] = attention(0, qt)
                normfinish(scs[0], 0, qt)
                if qt > 0:
                    outproj2(qt - 1, 0)
                    outproj2(qt - 1, 1)
                scs[1] = attention(1, qt)
                normfinish(scs[1], 1, qt)
            outproj2(QT - 1, 0)
            outproj2(QT - 1, 1)

    with _combined_exp_ln_tables():
        nc.compile()
    return nc


def vext(vt, pair, hl):
    """lhsT slice of the extended-V tile for (pair, local head hl).
    Both slices are 128 cols wide (FWL-eligible); the pad region of head 0
    is the constant filler (r lands on partition 64; partitions 65.. are
    junk that is never read)."""
    base = VSTR * pair
    if hl == 0:
        return vt[:, base : base + 128]         # V at 0-63, r at 64, pad
    return vt[:, base + 65 : base + VSTR]       # ones@32, V at 64-127


def make_mask():
    return (np.arange(P)[:, None] <= np.arange(P)[None, :]).astype(
        ml_dtypes.bfloat16)


def make_vpat():
    pat = np.zeros((P, 65), ml_dtypes.bfloat16)
    pat[:, 0] = 1.0   # even-head ones col (tile col 64): r -> partition 64
    pat[:, 33] = 1.0  # odd-head ones col (tile col 97): r -> partition 32
    return pat


def shard_inputs(x, w_qkv, b_qkv, w_out, L=2048):
    """Host-side sharding: core c = (batch c//4, head-group c%4)."""
    x = np.asarray(x, np.float32)
    w_qkv = np.asarray(w_qkv, np.float32)
    b_qkv = np.asarray(b_qkv, np.float32)
    w_out = np.asarray(w_out, np.float32)
    mask = make_mask()
    xTs = [np.ascontiguousarray(x[b].T.astype(ml_dtypes.bfloat16))
           for b in range(B)]
    in_maps = []
    for c in range(8):
        b, g = divmod(c, 4)
        qs, ks, vs = 256 * g, D + 256 * g, 2 * D + 256 * g
        wqk = np.ascontiguousarray(
            np.concatenate(
                [w_qkv[:, qs : qs + GD], w_qkv[:, ks : ks + GD]], axis=1
            ).astype(ml_dtypes.bfloat16)
        )
        wv = np.ascontiguousarray(
            w_qkv[:, vs : vs + GD].astype(ml_dtypes.bfloat16))
        wo = np.ascontiguousarray(
            w_out[256 * g : 256 * g + GD, :].astype(ml_dtypes.bfloat16))
        bqk = np.concatenate(
            [b_qkv[qs : qs + GD], b_qkv[ks : ks + GD]]
        ).reshape(2 * GD, 1).astype(np.float32)
        in_maps.append(
            {"xT": xTs[b], "wqk": wqk, "wv": wv, "wo": wo, "bqk": bqk,
             "mask": mask, "vpat": make_vpat()}
        )
    return in_maps


_NC_CACHE = {}


def get_nc(L=2048):
    if L not in _NC_CACHE:
        _NC_CACHE[L] = build_nc(L)
    return _NC_CACHE[L]


def gather(results, b_qkv, w_out, b_out, L=2048):
    fix = (np.asarray(b_qkv, np.float32)[2 * D :] @ np.asarray(w_out, np.float32)
           + np.asarray(b_out, np.float32))
    y = np.zeros((B, L, D), np.float32)
    for c in range(8):
        b = c // 4
        y[b] += results[c]["y"]
    y += fix[None, None, :]
    return y


def kernel(x, w_qkv, b_qkv, w_out, b_out):
    L = x.shape[1]
    nc = get_nc(L)
    in_maps = shard_inputs(x, w_qkv, b_qkv, w_out, L=L)
    res = run_bass_kernel_spmd(nc, in_maps, core_ids=list(range(8)))
    return gather(res.results, b_qkv, w_out, b_out, L=L)

